# revision 27
# baseline (speedup 1.0000x reference)
"""Trainium2 Bass kernel for nn_L3_31799937859925 (sparse_attention).

Strategy (v2 — algebraic folding + compensated-fp8 DoubleRow):

Math folding (host, weight-only — a compiler could do this):
  E = Cm @ w_up.T, F = Wmix_up_folded @ E.T, G+ = blockdiag(E_l E_l^T)/D_UP + eps.
  Then  up-rms   : |up|^2/D + eps*z^2 = pu^T G+ pu   (z colsum folds into G+)
        out      = rsqrt(pu^T G+ pu) * (F_win @ pu) + Wmix_x @ x
  so neither comb nor up is ever materialized. Device FLOPs per query drop
  from (W*H + W*D + D*D_UP + (D_UP+H)*H) to (W*H + W*W + H*W + H*H).

Sharding: queries sorted by label on host; 8 cores get 2048 contiguous
queries each (data parallel, no collectives). kv rows label-sorted so each
512-query tile sees a small contiguous window (W cols) + a 0/1 mask.

Precision: all big GEMMs run as compensated fp8-e4m3 DoubleRow matmuls
(A ~ A_hi + A_lo with A_lo = A - A_hi at the same scale, so hi/lo terms
accumulate in one PSUM group): hi@hi + hi@lo + lo@hi = 0.75x the bf16
instruction count at ~bf16+ accuracy. G-path runs bf16. Scales: K*2^5,
Wx*2^3, F*2^9, s_out*2^-6 (product = 2^3 on both PSUM paths; host divides
the bf16 output by 8). Measured emulation rel-err: 4.3e-3.
"""
import numpy as np
import ml_dtypes

import concourse.bass as bass
import concourse.tile as tile
from concourse import bacc, mybir
import concourse.bass_utils as bass_utils

F32 = mybir.dt.float32
F32R = mybir.dt.float32r
BF16 = mybir.dt.bfloat16
F8 = mybir.dt.float8e4
AF = mybir.ActivationFunctionType
MUL = mybir.AluOpType.mult
ADD = mybir.AluOpType.add
DR = mybir.MatmulPerfMode.DoubleRow

NPF8 = ml_dtypes.float8_e4m3
NPBF = ml_dtypes.bfloat16

H, N_EMB, D_EMB, D_UP = 1024, 8192, 512, 2048
B, T = 4, 4096
BT = B * T
NC = 8
NQ = BT // NC               # 2048 queries per core
QT = 512                    # queries per q-tile
NQT = NQ // QT              # 4 q-tiles per core
HC = H // 128               # 8
MC = H // 128               # 8 output chunks

EPS = 1e-6

LAST_RESULTS = None
LAST_EXEC_S = None
_PROGRAM_CACHE = {}


def _build_program(W):
    KV = W // 128
    nc = bacc.Bacc("TRN2", target_bir_lowering=False, debug=False,
                   enable_asserts=False)

    xc_in = nc.dram_tensor("xc_in", [NQT, 128, HC, 2, QT], F8, kind="ExternalInput")
    kt_in = nc.dram_tensor("kt_in", [NQT, 128, HC, 2, W], F8, kind="ExternalInput")
    wx_in = nc.dram_tensor("wx_in", [128, HC, 2, H], F8, kind="ExternalInput")
    f_in = nc.dram_tensor("f_in", [NQT, 128, KV, 2, H], F8, kind="ExternalInput")
    g_in = nc.dram_tensor("g_in", [NQT, 128, KV, W], BF16, kind="ExternalInput")
    mk_in = nc.dram_tensor("mk_in", [NQT, 128, KV, QT], BF16, kind="ExternalInput")
    out_d = nc.dram_tensor("out_d", [128, MC, NQ], BF16, kind="ExternalOutput")

    from contextlib import ExitStack
    with tile.TileContext(nc) as tc, ExitStack() as ctx, \
            nc.allow_low_precision("fp8/f32r compensated kernel"):
        ec = ctx.enter_context
        cst = ec(tc.tile_pool(name="cst", bufs=1))
        pwx = ec(tc.tile_pool(name="pwx", bufs=1))
        pxc = ec(tc.tile_pool(name="pxc", bufs=2))
        pkt = ec(tc.tile_pool(name="pkt", bufs=2))
        pg = ec(tc.tile_pool(name="pg", bufs=2))
        pf = ec(tc.tile_pool(name="pf", bufs=2))
        pmk = ec(tc.tile_pool(name="pmk", bufs=2))
        px2 = ec(tc.tile_pool(name="px2", bufs=3))
        pt = ec(tc.tile_pool(name="pt", bufs=4))
        ppu = ec(tc.tile_pool(name="ppu", bufs=3))
        pm = ec(tc.tile_pool(name="pm", bufs=3))
        pps = ec(tc.tile_pool(name="pps", bufs=3))
        prsb = ec(tc.tile_pool(name="prsb", bufs=3))
        pcs = ec(tc.tile_pool(name="pcs", bufs=4))
        pmsb = ec(tc.tile_pool(name="pmsb", bufs=2))
        ptmp = ec(tc.tile_pool(name="ptmp", bufs=3))
        po = ec(tc.tile_pool(name="po", bufs=2))
        # PSUM: s 1 + mix 2 + f/v 3 + row 1 + bc 1 = 8 banks
        psc = ec(tc.tile_pool(name="psc", bufs=1, space="PSUM"))
        pmix = ec(tc.tile_pool(name="pmix", bufs=2, space="PSUM"))
        pfp = ec(tc.tile_pool(name="pfp", bufs=3, space="PSUM"))
        prow = ec(tc.tile_pool(name="prow", bufs=1, space="PSUM"))
        pbc = ec(tc.tile_pool(name="pbc", bufs=1, space="PSUM"))

        ones_f = cst.tile([128, 2, 32], F32)
        nc.vector.memset(ones_f, 1.0)
        ones8 = cst.tile([128, 2, 32], F8)
        nc.vector.tensor_copy(ones8, ones_f)
        ones_bf = cst.tile([128, 1], BF16)
        nc.vector.tensor_copy(ones_bf, ones_f[:, 0, 0:1])
        ones_rf = cst.tile([1, 128], F32)
        nc.vector.memset(ones_rf, 1.0)
        ones_row = cst.tile([1, 128], F32R)
        nc.vector.tensor_copy(ones_row, ones_rf)
        eps_b = cst.tile([128, 1], F32)
        nc.vector.memset(eps_b, 1024.0 * EPS)
        zero_b = cst.tile([128, 1], F32)
        nc.vector.memset(zero_b, 0.0)

        wx_sb = pwx.tile([128, HC, 2, H], F8)

        def emit_mix(out_ps, wx_t, xc_t, mc):
            c0 = mc * 128
            for g in range(HC // 2):
                nc.tensor.matmul(out_ps, lhsT=wx_t[:, 2 * g:2 * g + 2, 0, c0:c0 + 128],
                                 rhs=xc_t[:, 2 * g:2 * g + 2, 1, :],
                                 start=(g == 0), stop=False, perf_mode=DR)
            for j in range(HC):
                nc.tensor.matmul(out_ps, lhsT=wx_t[:, j, 0:2, c0:c0 + 128],
                                 rhs=xc_t[:, j, 0:2, :],
                                 start=False, stop=(j == HC - 1), perf_mode=DR)

        def emit_scores(out_ps, kt_t, xc_t, kc):
            c0 = kc * 128
            for g in range(HC // 2):
                nc.tensor.matmul(out_ps, lhsT=kt_t[:, 2 * g:2 * g + 2, 0, c0:c0 + 128],
                                 rhs=xc_t[:, 2 * g:2 * g + 2, 1, :],
                                 start=(g == 0), stop=False, perf_mode=DR)
            for j in range(HC):
                nc.tensor.matmul(out_ps, lhsT=kt_t[:, j, 0:2, c0:c0 + 128],
                                 rhs=xc_t[:, j, 0:2, :],
                                 start=False, stop=(j == HC - 1), perf_mode=DR)

        def emit_F(out_ps, ft_t, ps_t, mc):
            c0 = mc * 128
            for g in range(KV // 2):
                nc.tensor.matmul(out_ps, lhsT=ft_t[:, 2 * g:2 * g + 2, 0, c0:c0 + 128],
                                 rhs=ps_t[:, 2 * g:2 * g + 2, 1, :],
                                 start=(g == 0), stop=False, perf_mode=DR)
            for i in range(KV):
                nc.tensor.matmul(out_ps, lhsT=ft_t[:, i, 0:2, c0:c0 + 128],
                                 rhs=ps_t[:, i, 0:2, :],
                                 start=False, stop=(i == KV - 1), perf_mode=DR)

        def stage_A(qt):
            ctx = {"qs": slice(qt * QT, (qt + 1) * QT)}
            xc_t = pxc.tile([128, HC, 2, QT], F8, tag="xc")
            kt_t = pkt.tile([128, HC, 2, W], F8, tag="kt")
            if qt == 0:
                nc.sync.dma_start(xc_t[:, :, 1, :], xc_in.ap()[qt][:, :, 1, :])
                nc.sync.dma_start(kt_t[:], kt_in.ap()[qt])
                nc.sync.dma_start(xc_t[:, :, 0, :], xc_in.ap()[qt][:, :, 0, :])
                nc.sync.dma_start(wx_a[:], wx_in.ap()[:, :, :, 0:H // 2])
            else:
                nc.sync.dma_start(xc_t[:], xc_in.ap()[qt])
                nc.sync.dma_start(kt_t[:], kt_in.ap()[qt])
            mk_t = pmk.tile([128, KV, QT], BF16, tag="mk")
            nc.sync.dma_start(mk_t[:], mk_in.ap()[qt])
            gt_t = pg.tile([128, KV, W], BF16, tag="g")
            nc.sync.dma_start(gt_t[:], g_in.ap()[qt])
            ft_t = pf.tile([128, KV, 2, H], F8, tag="f")
            nc.sync.dma_start(ft_t[:], f_in.ap()[qt])
            if qt == 0:
                nc.sync.dma_start(wx_b[:], wx_in.ap()[:, :, :, H // 2:H])
            ctx["xc"] = xc_t
            ctx["ft"] = ft_t

            # rms_in stats from x_hi squares (fp8 DoubleRow colsum)
            x2_a = px2.tile([128, 4, QT], F8, tag="x2a")
            nc.scalar.activation(x2_a[:, 0:2, :], xc_t[:, 0:2, 1, :], AF.Square, bias=zero_b)
            x2_b = px2.tile([128, 4, QT], F8, tag="x2b")
            nc.vector.tensor_tensor(x2_b[:, 0:2, :], xc_t[:, 4:6, 1, :], xc_t[:, 4:6, 1, :], MUL)
            nc.scalar.activation(x2_a[:, 2:4, :], xc_t[:, 2:4, 1, :], AF.Square, bias=zero_b)
            nc.vector.tensor_tensor(x2_b[:, 2:4, :], xc_t[:, 6:8, 1, :], xc_t[:, 6:8, 1, :], MUL)
            ss_ps = prow.tile([32, QT], F32, tag="row")
            for g in [0, 2, 1, 3]:
                x2h = x2_a if g < 2 else x2_b
                nc.tensor.matmul(ss_ps, lhsT=ones8, rhs=x2h[:, 2 * (g % 2):2 * (g % 2) + 2, :],
                                 start=(g == 0), stop=(g == 3),
                                 perf_mode=DR)
            sd = prsb.tile([1, QT], F32, tag="sd")
            nc.scalar.activation(sd, ss_ps[0:1, :], AF.Sqrt, bias=eps_b[:1],
                                 scale=1024.0 / H)
            nc.scalar.activation(dummy_e, zero_b[:1], AF.Exp, bias=zero_b[:1])
            crr = prsb.tile([1, QT], F32R, tag="crr")
            nc.vector.reciprocal(crr, sd)             # c * 2^-5
            cb_ps = pbc.tile([128, QT], F32, tag="bc")
            nc.tensor.matmul(cb_ps, lhsT=ones_row, rhs=crr,
                             start=True, stop=True)
            c_sb = pcs.tile([128, QT], BF16, tag="cs")
            nc.scalar.activation(c_sb, cb_ps, AF.Copy)

            pu_t = ppu.tile([128, KV, QT], BF16, tag="pu")
            ps_t = pps.tile([128, KV, 2, QT], F8, tag="ps")
            s_ps = psc.tile([128, QT], F32, tag="s")
            emit_scores(s_ps, kt_t, xc_t, 0)
            t_sb = pt.tile([128, QT], F32, tag="t")
            nc.vector.tensor_tensor(t_sb, s_ps, c_sb, MUL)
            nc.scalar.activation(pu_t[:, 0, :], t_sb, AF.Exp, bias=zero_b)
            nc.vector.tensor_tensor(pu_t[:, 0, :], pu_t[:, 0, :], mk_t[:, 0, :], MUL)
            s_ps2 = psc.tile([128, QT], F32, tag="s")
            emit_scores(s_ps2, kt_t, xc_t, 1)
            t_sb2 = pt.tile([128, QT], F32, tag="t")
            nc.vector.tensor_tensor(t_sb2, s_ps2, c_sb, MUL)
            nc.scalar.activation(pu_t[:, 1, :], t_sb2, AF.Exp, bias=zero_b)
            nc.vector.tensor_tensor(pu_t[:, 1, :], pu_t[:, 1, :], mk_t[:, 1, :], MUL)
            nc.scalar.activation(dummy_s, zero_b[:1], AF.Sqrt, bias=zero_b[:1])

            # hi/lo split of pu for the compensated F matmuls
            nc.gpsimd.tensor_copy(ps_t[:, :, 1, :], pu_t[:])
            nc.gpsimd.tensor_sub(ps_t[:, :, 0, :], pu_t[:], ps_t[:, :, 1, :])

            # G path: v = G+ @ pu ; qq = colsum(pu .* v)
            m_t = pm.tile([128, KV, QT], BF16, tag="m")
            for wc in range(KV):
                v_ps = pfp.tile([128, QT], F32, tag="f")
                for i in range(KV):
                    nc.tensor.matmul(v_ps, lhsT=gt_t[:, i, wc * 128:(wc + 1) * 128],
                                     rhs=pu_t[:, i, :],
                                     start=(i == 0), stop=(i == KV - 1))
                nc.vector.tensor_tensor(m_t[:, wc, :], pu_t[:, wc, :], v_ps, MUL)
            qq_ps = prow.tile([32, QT], F32, tag="row")
            for i in range(KV):
                nc.tensor.matmul(qq_ps[0:1, :], lhsT=ones_bf, rhs=m_t[:, i, :],
                                 start=(i == 0), stop=(i == KV - 1))
            sd2 = prsb.tile([1, QT], F32, tag="sd")
            nc.scalar.activation(sd2, qq_ps[0:1, :], AF.Sqrt, bias=zero_b[:1],
                                 scale=4096.0)
            sr2r = prsb.tile([1, QT], F32R, tag="crr")
            nc.vector.reciprocal(sr2r, sd2)           # s_out * 2^-6
            sb_ps = pbc.tile([128, QT], F32, tag="bc")
            nc.tensor.matmul(sb_ps, lhsT=ones_row, rhs=sr2r,
                             start=True, stop=True)
            s_sb = pcs.tile([128, QT], BF16, tag="cs")
            nc.scalar.activation(s_sb, sb_ps, AF.Copy)
            ctx["s_sb"] = s_sb
            ctx["ps"] = ps_t
            return ctx

        def start_mix(ctx, mc):
            mtile = pmix.tile([128, QT], F32, tag="mix")
            ctx["mix_ps"][mc] = mtile
            emit_mix(mtile, ctx["xc"], mc)

        def copy_mix(ctx, mc, dve=False):
            msb = pmsb.tile([128, QT], BF16, tag="msb")
            ctx["mix_sb"][mc] = msb
            if dve:
                nc.vector.tensor_copy(msb, ctx["mix_ps"][mc])
            else:
                nc.scalar.activation(msb, ctx["mix_ps"][mc], AF.Copy)

        def do_F(ctx, mc):
            if mc % 2 == 0:
                oq = po.tile([128, 2, QT], BF16, tag="o")
                ctx["o_sb"][mc // 2] = oq
            f_ps = pfp.tile([128, QT], F32, tag="f")
            emit_F(f_ps, ctx["ft"], ctx["ps"], mc)
            tmp = ptmp.tile([128, QT], BF16, tag="tmp")
            nc.vector.tensor_tensor(tmp, f_ps, ctx["s_sb"], MUL)
            eng = nc.gpsimd if mc % 2 == 0 else nc.vector
            eng.tensor_tensor(ctx["o_sb"][mc // 2][:, mc % 2, :], tmp,
                              ctx["mix_sb"][mc], ADD)

        def stage_B1(ctx):
            ctx["mix_sb"] = [None] * MC
            ctx["mix_ps"] = [None] * MC
            ctx["o_sb"] = [None] * 4
            for mc in range(4):
                start_mix(ctx, mc)
                copy_mix(ctx, mc)

        def stage_B2(ctx, last=False):
            qs = ctx["qs"]
            do_F(ctx, 0)
            start_mix(ctx, 4)
            copy_mix(ctx, 4)
            do_F(ctx, 1)
            nc.sync.dma_start(out_d.ap()[:, 0:2, qs], ctx["o_sb"][0][:])
            start_mix(ctx, 5)
            copy_mix(ctx, 5)
            do_F(ctx, 2)
            start_mix(ctx, 6)
            copy_mix(ctx, 6, dve=last)
            do_F(ctx, 3)
            nc.sync.dma_start(out_d.ap()[:, 2:4, qs], ctx["o_sb"][1][:])
            start_mix(ctx, 7)
            copy_mix(ctx, 7, dve=last)
            do_F(ctx, 4)
            do_F(ctx, 5)
            nc.sync.dma_start(out_d.ap()[:, 4:6, qs], ctx["o_sb"][2][:])
            do_F(ctx, 6)
            nc.sync.dma_start(out_d.ap()[:, 6:7, qs], ctx["o_sb"][3][:, 0:1, :])
            do_F(ctx, 7)
            nc.sync.dma_start(out_d.ap()[:, 7:8, qs], ctx["o_sb"][3][:, 1:2, :])

        cur = stage_A(0)
        for qt in range(NQT):
            stage_B1(cur)
            nxt = stage_A(qt + 1) if qt + 1 < NQT else None
            stage_B2(cur, last=(qt == NQT - 1))
            cur = nxt

    nc.compile()
    return nc


def _get_program(W):
    if W not in _PROGRAM_CACHE:
        _PROGRAM_CACHE[W] = _build_program(W)
    return _PROGRAM_CACHE[W]


def _f8(a):
    return np.asarray(a, NPF8)


def _hi_lo(a):
    hi = _f8(a)
    lo = _f8(a - hi.astype(np.float32))
    return hi, lo


def kernel(**inputs) -> np.ndarray:
    global LAST_RESULTS, LAST_EXEC_S
    inp = np.asarray(inputs["input"], np.float32)
    fw = np.asarray(inputs["fw"]).astype(np.int64)
    seq_sort = np.asarray(inputs["seq_sort"]).astype(np.int64)
    keep_cols = np.asarray(inputs["keep_cols"]).astype(np.int64)
    emb_alloc = np.asarray(inputs["emb_alloc"]).astype(np.int64)
    starts = np.asarray(inputs["starts"]).astype(np.int64)
    ends = np.asarray(inputs["ends"]).astype(np.int64)
    bb = int(np.asarray(inputs["bb"]))
    w_k = np.asarray(inputs["w_k_weight"], np.float32)
    w_v = np.asarray(inputs["w_v_weight"], np.float32)
    w_up = np.asarray(inputs["w_up_weight"], np.float32)
    w_mix = np.asarray(inputs["w_mix_weight"], np.float32)
    w_in = np.asarray(inputs["norm_in_weight"], np.float32)
    w_out = np.asarray(inputs["norm_out_weight"], np.float32)

    x = inp.reshape(BT, H)
    nb = BT // bb
    st = starts.reshape(nb, bb).min(axis=1)
    en = ends.reshape(nb, bb).max(axis=1)

    # sort queries by label; row s of sorted space is original query perm[s]
    order = np.argsort(seq_sort, kind="stable")
    perm = fw[order]
    lab_q = seq_sort[order]
    blk_q = order // bb
    st_q = st[blk_q]
    en_q = en[blk_q]
    x_sorted = x[perm]                       # [BT, H]

    # kv side: keep + label-sort; fold norm_in into K
    la = emb_alloc[keep_cols]                # [M]
    M = la.shape[0]
    kv_order = np.argsort(la, kind="stable")
    la_s = la[kv_order]
    kvpos = kv_order
    Bm = (w_k[keep_cols] * w_in[None, :])[kv_order]   # [M, H]
    Cm = w_v[keep_cols][kv_order]            # [M, D_EMB]

    counts = np.bincount(la_s, minlength=64)
    gstart = np.concatenate([[0], np.cumsum(counts)])

    NT = BT // QT                            # 32 global q-tiles
    win = np.empty(NT, np.int64)
    need = 0
    for g in range(NT):
        l0 = lab_q[g * QT]
        l1 = lab_q[(g + 1) * QT - 1]
        win[g] = gstart[l0]
        need = max(need, gstart[l1 + 1] - gstart[l0])
    W = max(256, int(-(-need // 256) * 256))
    KV = W // 128

    Mp = M + W
    Bm_p = np.zeros((Mp, H), np.float32); Bm_p[:M] = Bm
    Cm_p = np.zeros((Mp, D_EMB), np.float32); Cm_p[:M] = Cm
    la_p = np.full(Mp, -1, np.int64); la_p[:M] = la_s
    kvpos_p = np.full(Mp, -1, np.int64); kvpos_p[:M] = kvpos

    # ---- folded weights (weight-only precompute)
    Wmix = w_mix.copy()
    Wmix[:, :D_UP] *= w_out[None, :]
    Wmix_up = Wmix[:, :D_UP]                 # [H, D_UP]
    Wmix_x = np.ascontiguousarray(Wmix[:, D_UP:])  # [H, H]
    P1 = Wmix_up @ w_up                      # [H, D_EMB]
    F_full = P1 @ Cm_p.T                     # [H, Mp]
    QQ = w_up.T @ w_up                       # [D_EMB, D_EMB]
    G_full = np.zeros((Mp, Mp), np.float32)
    for l in range(64):
        a, b = int(gstart[l]), int(gstart[l + 1])
        if b > a:
            Cl = Cm_p[a:b]
            G_full[a:b, a:b] = (Cl @ QQ) @ Cl.T

    K_hi, K_lo = _hi_lo(Bm_p * 32.0)                       # [Mp, H]
    Wx_hi, Wx_lo = _hi_lo(Wmix_x * 8.0)                    # [H, H]
    F_hi, F_lo = _hi_lo(F_full * 512.0)                    # [H, Mp]

    # mask: label match + [start, end) on kept positions
    kvi = win[:, None] + np.arange(W)[None, :]             # [NT, W]
    la_w = la_p[kvi]
    kp_w = kvpos_p[kvi]
    lab_t = lab_q.reshape(NT, QT)
    st_t = st_q.reshape(NT, QT)
    en_t = en_q.reshape(NT, QT)
    valid = ((la_w[:, None, :] == lab_t[:, :, None])
             & (kp_w[:, None, :] >= st_t[:, :, None])
             & (kp_w[:, None, :] < en_t[:, :, None]))      # [NT, QT, W]
    mask01 = valid.astype(NPBF)

    def chunk_pT(arr2d, nchunk):
        # [rows, cols] -> [128, nchunk, cols] with rows = nchunk*128
        return np.ascontiguousarray(
            arr2d.reshape(nchunk, 128, arr2d.shape[1]).transpose(1, 0, 2))

    # static: wx host layout [128, HC, 2, H]
    wx_host = np.empty((128, HC, 2, H), NPF8)
    wx_host[:, :, 0, :] = chunk_pT(np.ascontiguousarray(Wx_hi.T), HC)
    wx_host[:, :, 1, :] = chunk_pT(np.ascontiguousarray(Wx_lo.T), HC)

    in_maps = []
    for c in range(NC):
        xc_c = np.empty((NQT, 128, HC, 2, QT), NPF8)
        kt_c = np.empty((NQT, 128, HC, 2, W), NPF8)
        f_c = np.empty((NQT, 128, KV, 2, H), NPF8)
        g_c = np.empty((NQT, 128, KV, W), NPBF)
        mk_c = np.empty((NQT, 128, KV, QT), NPBF)
        for qt in range(NQT):
            g = c * NQT + qt
            w0 = int(win[g])
            qs = slice(g * QT, (g + 1) * QT)
            Xt = np.ascontiguousarray(x_sorted[qs].T)      # [H, QT]
            xhi = _f8(Xt)
            xlo = _f8(Xt - xhi.astype(np.float32))
            xc_c[qt, :, :, 0, :] = chunk_pT(xlo, HC)
            xc_c[qt, :, :, 1, :] = chunk_pT(xhi, HC)
            kt_c[qt, :, :, 0, :] = chunk_pT(
                np.ascontiguousarray(K_hi[w0:w0 + W].astype(np.float32).T).astype(NPF8), HC)
            kt_c[qt, :, :, 1, :] = chunk_pT(
                np.ascontiguousarray(K_lo[w0:w0 + W].astype(np.float32).T).astype(NPF8), HC)
            f_c[qt, :, :, 0, :] = chunk_pT(
                np.ascontiguousarray(F_hi[:, w0:w0 + W].astype(np.float32).T).astype(NPF8), KV)
            f_c[qt, :, :, 1, :] = chunk_pT(
                np.ascontiguousarray(F_lo[:, w0:w0 + W].astype(np.float32).T).astype(NPF8), KV)
            Gw = (G_full[w0:w0 + W, w0:w0 + W] * (1.0 / D_UP) + EPS).astype(NPBF)
            g_c[qt] = chunk_pT(Gw, KV)
            mk_c[qt] = chunk_pT(np.ascontiguousarray(mask01[g].T), KV)
        in_maps.append({
            "xc_in": xc_c, "kt_in": kt_c, "wx_in": wx_host,
            "f_in": f_c, "g_in": g_c, "mk_in": mk_c,
        })

    ncprog = _get_program(W)
    import time as _time
    _t0 = _time.time()
    LAST_RESULTS = bass_utils.run_bass_kernel_spmd(ncprog, in_maps,
                                                   core_ids=list(range(NC)))
    LAST_EXEC_S = _time.time() - _t0
    # out_d [128, MC, NQ] bf16 holds 8*out
    out_sorted = np.concatenate(
        [np.asarray(r["out_d"], NPBF).astype(np.float32)
         .transpose(1, 0, 2).reshape(H, NQ).T
         for r in LAST_RESULTS.results], axis=0) * 0.125   # [BT, H]
    final = np.empty((BT, H), np.float32)
    final[perm] = out_sorted
    return final.reshape(B, T, H)


# revision 28
# speedup vs baseline: 1.0053x; 1.0053x over previous
"""Trainium2 Bass kernel for nn_L3_31799937859925 (sparse_attention).

Strategy (v2 — algebraic folding + compensated-fp8 DoubleRow):

Math folding (host, weight-only — a compiler could do this):
  E = Cm @ w_up.T, F = Wmix_up_folded @ E.T, G+ = blockdiag(E_l E_l^T)/D_UP + eps.
  Then  up-rms   : |up|^2/D + eps*z^2 = pu^T G+ pu   (z colsum folds into G+)
        out      = rsqrt(pu^T G+ pu) * (F_win @ pu) + Wmix_x @ x
  so neither comb nor up is ever materialized. Device FLOPs per query drop
  from (W*H + W*D + D*D_UP + (D_UP+H)*H) to (W*H + W*W + H*W + H*H).

Sharding: queries sorted by label on host; 8 cores get 2048 contiguous
queries each (data parallel, no collectives). kv rows label-sorted so each
512-query tile sees a small contiguous window (W cols) + a 0/1 mask.

Precision: all big GEMMs run as compensated fp8-e4m3 DoubleRow matmuls
(A ~ A_hi + A_lo with A_lo = A - A_hi at the same scale, so hi/lo terms
accumulate in one PSUM group): hi@hi + hi@lo + lo@hi = 0.75x the bf16
instruction count at ~bf16+ accuracy. G-path runs bf16. Scales: K*2^5,
Wx*2^3, F*2^9, s_out*2^-6 (product = 2^3 on both PSUM paths; host divides
the bf16 output by 8). Measured emulation rel-err: 4.3e-3.
"""
import numpy as np
import ml_dtypes

import concourse.bass as bass
import concourse.tile as tile
from concourse import bacc, mybir
import concourse.bass_utils as bass_utils

F32 = mybir.dt.float32
F32R = mybir.dt.float32r
BF16 = mybir.dt.bfloat16
F8 = mybir.dt.float8e4
AF = mybir.ActivationFunctionType
MUL = mybir.AluOpType.mult
ADD = mybir.AluOpType.add
DR = mybir.MatmulPerfMode.DoubleRow

NPF8 = ml_dtypes.float8_e4m3
NPBF = ml_dtypes.bfloat16

H, N_EMB, D_EMB, D_UP = 1024, 8192, 512, 2048
B, T = 4, 4096
BT = B * T
NC = 8
NQ = BT // NC               # 2048 queries per core
QT = 512                    # queries per q-tile
NQT = NQ // QT              # 4 q-tiles per core
HC = H // 128               # 8
MC = H // 128               # 8 output chunks

EPS = 1e-6

LAST_RESULTS = None
LAST_EXEC_S = None
_PROGRAM_CACHE = {}


def _build_program(W):
    KV = W // 128
    nc = bacc.Bacc("TRN2", target_bir_lowering=False, debug=False,
                   enable_asserts=False)

    xc_in = nc.dram_tensor("xc_in", [NQT, 128, HC, 2, QT], F8, kind="ExternalInput")
    kt_in = nc.dram_tensor("kt_in", [NQT, 128, HC, 2, W], F8, kind="ExternalInput")
    wx_in = nc.dram_tensor("wx_in", [128, HC, 2, H], F8, kind="ExternalInput")
    f_in = nc.dram_tensor("f_in", [NQT, 128, KV, 2, H], F8, kind="ExternalInput")
    g_in = nc.dram_tensor("g_in", [NQT, 128, KV, W], BF16, kind="ExternalInput")
    mk_in = nc.dram_tensor("mk_in", [NQT, 128, KV, QT], BF16, kind="ExternalInput")
    out_d = nc.dram_tensor("out_d", [128, MC, NQ], BF16, kind="ExternalOutput")

    from contextlib import ExitStack
    with tile.TileContext(nc) as tc, ExitStack() as ctx, \
            nc.allow_low_precision("fp8/f32r compensated kernel"):
        ec = ctx.enter_context
        cst = ec(tc.tile_pool(name="cst", bufs=1))
        pwx = ec(tc.tile_pool(name="pwx", bufs=1))
        pxc = ec(tc.tile_pool(name="pxc", bufs=2))
        pkt = ec(tc.tile_pool(name="pkt", bufs=2))
        pg = ec(tc.tile_pool(name="pg", bufs=2))
        pf = ec(tc.tile_pool(name="pf", bufs=2))
        pmk = ec(tc.tile_pool(name="pmk", bufs=2))
        px2 = ec(tc.tile_pool(name="px2", bufs=3))
        pt = ec(tc.tile_pool(name="pt", bufs=4))
        ppu = ec(tc.tile_pool(name="ppu", bufs=3))
        pm = ec(tc.tile_pool(name="pm", bufs=3))
        pps = ec(tc.tile_pool(name="pps", bufs=3))
        prsb = ec(tc.tile_pool(name="prsb", bufs=3))
        pcs = ec(tc.tile_pool(name="pcs", bufs=4))
        pmsb = ec(tc.tile_pool(name="pmsb", bufs=2))
        ptmp = ec(tc.tile_pool(name="ptmp", bufs=3))
        po = ec(tc.tile_pool(name="po", bufs=2))
        # PSUM: s 1 + mix 2 + f/v 3 + row 1 + bc 1 = 8 banks
        psc = ec(tc.tile_pool(name="psc", bufs=1, space="PSUM"))
        pmix = ec(tc.tile_pool(name="pmix", bufs=2, space="PSUM"))
        pfp = ec(tc.tile_pool(name="pfp", bufs=3, space="PSUM"))
        prow = ec(tc.tile_pool(name="prow", bufs=1, space="PSUM"))
        pbc = ec(tc.tile_pool(name="pbc", bufs=1, space="PSUM"))

        ones_f = cst.tile([128, 2, 32], F32)
        nc.vector.memset(ones_f, 1.0)
        ones8 = cst.tile([128, 2, 32], F8)
        nc.vector.tensor_copy(ones8, ones_f)
        ones_bf = cst.tile([128, 1], BF16)
        nc.vector.tensor_copy(ones_bf, ones_f[:, 0, 0:1])
        ones_rf = cst.tile([1, 128], F32)
        nc.vector.memset(ones_rf, 1.0)
        ones_row = cst.tile([1, 128], F32R)
        nc.vector.tensor_copy(ones_row, ones_rf)
        eps_b = cst.tile([128, 1], F32)
        nc.vector.memset(eps_b, 1024.0 * EPS)
        zero_b = cst.tile([128, 1], F32)
        nc.vector.memset(zero_b, 0.0)

        wx_sb = pwx.tile([128, HC, 2, H], F8)

        def emit_mix(out_ps, wx_t, xc_t, mc):
            c0 = mc * 128
            for g in range(HC // 2):
                nc.tensor.matmul(out_ps, lhsT=wx_t[:, 2 * g:2 * g + 2, 0, c0:c0 + 128],
                                 rhs=xc_t[:, 2 * g:2 * g + 2, 1, :],
                                 start=(g == 0), stop=False, perf_mode=DR)
            for j in range(HC):
                nc.tensor.matmul(out_ps, lhsT=wx_t[:, j, 0:2, c0:c0 + 128],
                                 rhs=xc_t[:, j, 0:2, :],
                                 start=False, stop=(j == HC - 1), perf_mode=DR)

        def emit_scores(out_ps, kt_t, xc_t, kc):
            c0 = kc * 128
            for g in range(HC // 2):
                nc.tensor.matmul(out_ps, lhsT=kt_t[:, 2 * g:2 * g + 2, 0, c0:c0 + 128],
                                 rhs=xc_t[:, 2 * g:2 * g + 2, 1, :],
                                 start=(g == 0), stop=False, perf_mode=DR)
            for j in range(HC):
                nc.tensor.matmul(out_ps, lhsT=kt_t[:, j, 0:2, c0:c0 + 128],
                                 rhs=xc_t[:, j, 0:2, :],
                                 start=False, stop=(j == HC - 1), perf_mode=DR)

        def emit_F(out_ps, ft_t, ps_t, mc):
            c0 = mc * 128
            for g in range(KV // 2):
                nc.tensor.matmul(out_ps, lhsT=ft_t[:, 2 * g:2 * g + 2, 0, c0:c0 + 128],
                                 rhs=ps_t[:, 2 * g:2 * g + 2, 1, :],
                                 start=(g == 0), stop=False, perf_mode=DR)
            for i in range(KV):
                nc.tensor.matmul(out_ps, lhsT=ft_t[:, i, 0:2, c0:c0 + 128],
                                 rhs=ps_t[:, i, 0:2, :],
                                 start=False, stop=(i == KV - 1), perf_mode=DR)

        def stage_A(qt):
            ctx = {"qs": slice(qt * QT, (qt + 1) * QT)}
            xc_t = pxc.tile([128, HC, 2, QT], F8, tag="xc")
            kt_t = pkt.tile([128, HC, 2, W], F8, tag="kt")
            if qt == 0:
                nc.sync.dma_start(xc_t[:, :, 1, :], xc_in.ap()[qt][:, :, 1, :])
                nc.sync.dma_start(kt_t[:], kt_in.ap()[qt])
                nc.sync.dma_start(xc_t[:, :, 0, :], xc_in.ap()[qt][:, :, 0, :])
                nc.sync.dma_start(wx_a[:], wx_in.ap()[:, :, :, 0:H // 2])
            else:
                nc.sync.dma_start(xc_t[:], xc_in.ap()[qt])
                nc.sync.dma_start(kt_t[:], kt_in.ap()[qt])
            mk_t = pmk.tile([128, KV, QT], BF16, tag="mk")
            nc.sync.dma_start(mk_t[:], mk_in.ap()[qt])
            gt_t = pg.tile([128, KV, W], BF16, tag="g")
            nc.sync.dma_start(gt_t[:], g_in.ap()[qt])
            ft_t = pf.tile([128, KV, 2, H], F8, tag="f")
            nc.sync.dma_start(ft_t[:], f_in.ap()[qt])
            if qt == 0:
                nc.sync.dma_start(wx_b[:], wx_in.ap()[:, :, :, H // 2:H])
            ctx["xc"] = xc_t
            ctx["ft"] = ft_t

            # rms_in stats from x_hi squares (fp8 DoubleRow colsum)
            x2_a = px2.tile([128, 4, QT], F8, tag="x2a")
            nc.scalar.activation(x2_a[:], xc_t[:, 0:4, 1, :], AF.Square, bias=zero_b)
            x2_b = px2.tile([128, 4, QT], F8, tag="x2b")
            nc.vector.tensor_tensor(x2_b[:], xc_t[:, 4:8, 1, :], xc_t[:, 4:8, 1, :], MUL)
            ss_ps = prow.tile([32, QT], F32, tag="row")
            for g in range(HC // 2):
                x2h = x2_a if g < 2 else x2_b
                nc.tensor.matmul(ss_ps, lhsT=ones8, rhs=x2h[:, 2 * (g % 2):2 * (g % 2) + 2, :],
                                 start=(g == 0), stop=(g == HC // 2 - 1),
                                 perf_mode=DR)
            sd = prsb.tile([1, QT], F32, tag="sd")
            nc.scalar.activation(sd, ss_ps[0:1, :], AF.Sqrt, bias=eps_b[:1],
                                 scale=1024.0 / H)
            nc.scalar.activation(dummy_e, zero_b[:1], AF.Exp, bias=zero_b[:1])
            crr = prsb.tile([1, QT], F32R, tag="crr")
            nc.vector.reciprocal(crr, sd)             # c * 2^-5
            cb_ps = pbc.tile([128, QT], F32, tag="bc")
            nc.tensor.matmul(cb_ps, lhsT=ones_row, rhs=crr,
                             start=True, stop=True)
            c_sb = pcs.tile([128, QT], BF16, tag="cs")
            nc.scalar.activation(c_sb, cb_ps, AF.Copy)

            pu_t = ppu.tile([128, KV, QT], BF16, tag="pu")
            ps_t = pps.tile([128, KV, 2, QT], F8, tag="ps")
            s_ps = psc.tile([128, QT], F32, tag="s")
            emit_scores(s_ps, kt_t, xc_t, 0)
            t_sb = pt.tile([128, QT], F32, tag="t")
            nc.vector.tensor_tensor(t_sb, s_ps, c_sb, MUL)
            nc.scalar.activation(pu_t[:, 0, :], t_sb, AF.Exp, bias=zero_b)
            nc.vector.tensor_tensor(pu_t[:, 0, :], pu_t[:, 0, :], mk_t[:, 0, :], MUL)
            s_ps2 = psc.tile([128, QT], F32, tag="s")
            emit_scores(s_ps2, kt_t, xc_t, 1)
            t_sb2 = pt.tile([128, QT], F32, tag="t")
            nc.vector.tensor_tensor(t_sb2, s_ps2, c_sb, MUL)
            nc.scalar.activation(pu_t[:, 1, :], t_sb2, AF.Exp, bias=zero_b)
            nc.vector.tensor_tensor(pu_t[:, 1, :], pu_t[:, 1, :], mk_t[:, 1, :], MUL)
            nc.scalar.activation(dummy_s, zero_b[:1], AF.Sqrt, bias=zero_b[:1])

            # hi/lo split of pu for the compensated F matmuls
            nc.gpsimd.tensor_copy(ps_t[:, :, 1, :], pu_t[:])
            nc.gpsimd.tensor_sub(ps_t[:, :, 0, :], pu_t[:], ps_t[:, :, 1, :])

            # G path: v = G+ @ pu ; qq = colsum(pu .* v)
            m_t = pm.tile([128, KV, QT], BF16, tag="m")
            for wc in range(KV):
                v_ps = pfp.tile([128, QT], F32, tag="f")
                for i in range(KV):
                    nc.tensor.matmul(v_ps, lhsT=gt_t[:, i, wc * 128:(wc + 1) * 128],
                                     rhs=pu_t[:, i, :],
                                     start=(i == 0), stop=(i == KV - 1))
                nc.vector.tensor_tensor(m_t[:, wc, :], pu_t[:, wc, :], v_ps, MUL)
            qq_ps = prow.tile([32, QT], F32, tag="row")
            for i in range(KV):
                nc.tensor.matmul(qq_ps[0:1, :], lhsT=ones_bf, rhs=m_t[:, i, :],
                                 start=(i == 0), stop=(i == KV - 1))
            sd2 = prsb.tile([1, QT], F32, tag="sd")
            nc.scalar.activation(sd2, qq_ps[0:1, :], AF.Sqrt, bias=zero_b[:1],
                                 scale=4096.0)
            sr2r = prsb.tile([1, QT], F32R, tag="crr")
            nc.vector.reciprocal(sr2r, sd2)           # s_out * 2^-6
            sb_ps = pbc.tile([128, QT], F32, tag="bc")
            nc.tensor.matmul(sb_ps, lhsT=ones_row, rhs=sr2r,
                             start=True, stop=True)
            s_sb = pcs.tile([128, QT], BF16, tag="cs")
            nc.scalar.activation(s_sb, sb_ps, AF.Copy)
            ctx["s_sb"] = s_sb
            ctx["ps"] = ps_t
            return ctx

        def start_mix(ctx, mc):
            mtile = pmix.tile([128, QT], F32, tag="mix")
            ctx["mix_ps"][mc] = mtile
            emit_mix(mtile, ctx["xc"], mc)

        def copy_mix(ctx, mc, dve=False):
            msb = pmsb.tile([128, QT], BF16, tag="msb")
            ctx["mix_sb"][mc] = msb
            if dve:
                nc.vector.tensor_copy(msb, ctx["mix_ps"][mc])
            else:
                nc.scalar.activation(msb, ctx["mix_ps"][mc], AF.Copy)

        def do_F(ctx, mc):
            if mc % 2 == 0:
                oq = po.tile([128, 2, QT], BF16, tag="o")
                ctx["o_sb"][mc // 2] = oq
            f_ps = pfp.tile([128, QT], F32, tag="f")
            emit_F(f_ps, ctx["ft"], ctx["ps"], mc)
            tmp = ptmp.tile([128, QT], BF16, tag="tmp")
            nc.vector.tensor_tensor(tmp, f_ps, ctx["s_sb"], MUL)
            eng = nc.gpsimd if mc % 2 == 0 else nc.vector
            eng.tensor_tensor(ctx["o_sb"][mc // 2][:, mc % 2, :], tmp,
                              ctx["mix_sb"][mc], ADD)

        def stage_B1(ctx):
            ctx["mix_sb"] = [None] * MC
            ctx["mix_ps"] = [None] * MC
            ctx["o_sb"] = [None] * 4
            for mc in range(4):
                start_mix(ctx, mc)
                copy_mix(ctx, mc)

        def stage_B2(ctx, last=False):
            qs = ctx["qs"]
            do_F(ctx, 0)
            start_mix(ctx, 4)
            copy_mix(ctx, 4)
            do_F(ctx, 1)
            nc.sync.dma_start(out_d.ap()[:, 0:2, qs], ctx["o_sb"][0][:])
            start_mix(ctx, 5)
            copy_mix(ctx, 5)
            do_F(ctx, 2)
            start_mix(ctx, 6)
            copy_mix(ctx, 6, dve=last)
            do_F(ctx, 3)
            nc.sync.dma_start(out_d.ap()[:, 2:4, qs], ctx["o_sb"][1][:])
            start_mix(ctx, 7)
            copy_mix(ctx, 7, dve=last)
            do_F(ctx, 4)
            do_F(ctx, 5)
            nc.sync.dma_start(out_d.ap()[:, 4:6, qs], ctx["o_sb"][2][:])
            do_F(ctx, 6)
            nc.sync.dma_start(out_d.ap()[:, 6:7, qs], ctx["o_sb"][3][:, 0:1, :])
            do_F(ctx, 7)
            nc.sync.dma_start(out_d.ap()[:, 7:8, qs], ctx["o_sb"][3][:, 1:2, :])

        cur = stage_A(0)
        for qt in range(NQT):
            stage_B1(cur)
            nxt = stage_A(qt + 1) if qt + 1 < NQT else None
            stage_B2(cur, last=(qt == NQT - 1))
            cur = nxt

    nc.compile()
    return nc


def _get_program(W):
    if W not in _PROGRAM_CACHE:
        _PROGRAM_CACHE[W] = _build_program(W)
    return _PROGRAM_CACHE[W]


def _f8(a):
    return np.asarray(a, NPF8)


def _hi_lo(a):
    hi = _f8(a)
    lo = _f8(a - hi.astype(np.float32))
    return hi, lo


def kernel(**inputs) -> np.ndarray:
    global LAST_RESULTS, LAST_EXEC_S
    inp = np.asarray(inputs["input"], np.float32)
    fw = np.asarray(inputs["fw"]).astype(np.int64)
    seq_sort = np.asarray(inputs["seq_sort"]).astype(np.int64)
    keep_cols = np.asarray(inputs["keep_cols"]).astype(np.int64)
    emb_alloc = np.asarray(inputs["emb_alloc"]).astype(np.int64)
    starts = np.asarray(inputs["starts"]).astype(np.int64)
    ends = np.asarray(inputs["ends"]).astype(np.int64)
    bb = int(np.asarray(inputs["bb"]))
    w_k = np.asarray(inputs["w_k_weight"], np.float32)
    w_v = np.asarray(inputs["w_v_weight"], np.float32)
    w_up = np.asarray(inputs["w_up_weight"], np.float32)
    w_mix = np.asarray(inputs["w_mix_weight"], np.float32)
    w_in = np.asarray(inputs["norm_in_weight"], np.float32)
    w_out = np.asarray(inputs["norm_out_weight"], np.float32)

    x = inp.reshape(BT, H)
    nb = BT // bb
    st = starts.reshape(nb, bb).min(axis=1)
    en = ends.reshape(nb, bb).max(axis=1)

    # sort queries by label; row s of sorted space is original query perm[s]
    order = np.argsort(seq_sort, kind="stable")
    perm = fw[order]
    lab_q = seq_sort[order]
    blk_q = order // bb
    st_q = st[blk_q]
    en_q = en[blk_q]
    x_sorted = x[perm]                       # [BT, H]

    # kv side: keep + label-sort; fold norm_in into K
    la = emb_alloc[keep_cols]                # [M]
    M = la.shape[0]
    kv_order = np.argsort(la, kind="stable")
    la_s = la[kv_order]
    kvpos = kv_order
    Bm = (w_k[keep_cols] * w_in[None, :])[kv_order]   # [M, H]
    Cm = w_v[keep_cols][kv_order]            # [M, D_EMB]

    counts = np.bincount(la_s, minlength=64)
    gstart = np.concatenate([[0], np.cumsum(counts)])

    NT = BT // QT                            # 32 global q-tiles
    win = np.empty(NT, np.int64)
    need = 0
    for g in range(NT):
        l0 = lab_q[g * QT]
        l1 = lab_q[(g + 1) * QT - 1]
        win[g] = gstart[l0]
        need = max(need, gstart[l1 + 1] - gstart[l0])
    W = max(256, int(-(-need // 256) * 256))
    KV = W // 128

    Mp = M + W
    Bm_p = np.zeros((Mp, H), np.float32); Bm_p[:M] = Bm
    Cm_p = np.zeros((Mp, D_EMB), np.float32); Cm_p[:M] = Cm
    la_p = np.full(Mp, -1, np.int64); la_p[:M] = la_s
    kvpos_p = np.full(Mp, -1, np.int64); kvpos_p[:M] = kvpos

    # ---- folded weights (weight-only precompute)
    Wmix = w_mix.copy()
    Wmix[:, :D_UP] *= w_out[None, :]
    Wmix_up = Wmix[:, :D_UP]                 # [H, D_UP]
    Wmix_x = np.ascontiguousarray(Wmix[:, D_UP:])  # [H, H]
    P1 = Wmix_up @ w_up                      # [H, D_EMB]
    F_full = P1 @ Cm_p.T                     # [H, Mp]
    QQ = w_up.T @ w_up                       # [D_EMB, D_EMB]
    G_full = np.zeros((Mp, Mp), np.float32)
    for l in range(64):
        a, b = int(gstart[l]), int(gstart[l + 1])
        if b > a:
            Cl = Cm_p[a:b]
            G_full[a:b, a:b] = (Cl @ QQ) @ Cl.T

    K_hi, K_lo = _hi_lo(Bm_p * 32.0)                       # [Mp, H]
    Wx_hi, Wx_lo = _hi_lo(Wmix_x * 8.0)                    # [H, H]
    F_hi, F_lo = _hi_lo(F_full * 512.0)                    # [H, Mp]

    # mask: label match + [start, end) on kept positions
    kvi = win[:, None] + np.arange(W)[None, :]             # [NT, W]
    la_w = la_p[kvi]
    kp_w = kvpos_p[kvi]
    lab_t = lab_q.reshape(NT, QT)
    st_t = st_q.reshape(NT, QT)
    en_t = en_q.reshape(NT, QT)
    valid = ((la_w[:, None, :] == lab_t[:, :, None])
             & (kp_w[:, None, :] >= st_t[:, :, None])
             & (kp_w[:, None, :] < en_t[:, :, None]))      # [NT, QT, W]
    mask01 = valid.astype(NPBF)

    def chunk_pT(arr2d, nchunk):
        # [rows, cols] -> [128, nchunk, cols] with rows = nchunk*128
        return np.ascontiguousarray(
            arr2d.reshape(nchunk, 128, arr2d.shape[1]).transpose(1, 0, 2))

    # static: wx host layout [128, HC, 2, H]
    wx_host = np.empty((128, HC, 2, H), NPF8)
    wx_host[:, :, 0, :] = chunk_pT(np.ascontiguousarray(Wx_hi.T), HC)
    wx_host[:, :, 1, :] = chunk_pT(np.ascontiguousarray(Wx_lo.T), HC)

    in_maps = []
    for c in range(NC):
        xc_c = np.empty((NQT, 128, HC, 2, QT), NPF8)
        kt_c = np.empty((NQT, 128, HC, 2, W), NPF8)
        f_c = np.empty((NQT, 128, KV, 2, H), NPF8)
        g_c = np.empty((NQT, 128, KV, W), NPBF)
        mk_c = np.empty((NQT, 128, KV, QT), NPBF)
        for qt in range(NQT):
            g = c * NQT + qt
            w0 = int(win[g])
            qs = slice(g * QT, (g + 1) * QT)
            Xt = np.ascontiguousarray(x_sorted[qs].T)      # [H, QT]
            xhi = _f8(Xt)
            xlo = _f8(Xt - xhi.astype(np.float32))
            xc_c[qt, :, :, 0, :] = chunk_pT(xlo, HC)
            xc_c[qt, :, :, 1, :] = chunk_pT(xhi, HC)
            kt_c[qt, :, :, 0, :] = chunk_pT(
                np.ascontiguousarray(K_hi[w0:w0 + W].astype(np.float32).T).astype(NPF8), HC)
            kt_c[qt, :, :, 1, :] = chunk_pT(
                np.ascontiguousarray(K_lo[w0:w0 + W].astype(np.float32).T).astype(NPF8), HC)
            f_c[qt, :, :, 0, :] = chunk_pT(
                np.ascontiguousarray(F_hi[:, w0:w0 + W].astype(np.float32).T).astype(NPF8), KV)
            f_c[qt, :, :, 1, :] = chunk_pT(
                np.ascontiguousarray(F_lo[:, w0:w0 + W].astype(np.float32).T).astype(NPF8), KV)
            Gw = (G_full[w0:w0 + W, w0:w0 + W] * (1.0 / D_UP) + EPS).astype(NPBF)
            g_c[qt] = chunk_pT(Gw, KV)
            mk_c[qt] = chunk_pT(np.ascontiguousarray(mask01[g].T), KV)
        in_maps.append({
            "xc_in": xc_c, "kt_in": kt_c, "wx_in": wx_host,
            "f_in": f_c, "g_in": g_c, "mk_in": mk_c,
        })

    ncprog = _get_program(W)
    import time as _time
    _t0 = _time.time()
    LAST_RESULTS = bass_utils.run_bass_kernel_spmd(ncprog, in_maps,
                                                   core_ids=list(range(NC)))
    LAST_EXEC_S = _time.time() - _t0
    # out_d [128, MC, NQ] bf16 holds 8*out
    out_sorted = np.concatenate(
        [np.asarray(r["out_d"], NPBF).astype(np.float32)
         .transpose(1, 0, 2).reshape(H, NQ).T
         for r in LAST_RESULTS.results], axis=0) * 0.125   # [BT, H]
    final = np.empty((BT, H), np.float32)
    final[perm] = out_sorted
    return final.reshape(B, T, H)
